# revision 14
# baseline (speedup 1.0000x reference)
"""Trainium2 Bass kernel for nn_Attention_gvtn (8-head spatial attention,
softmax over the query axis), distributed over 8 NeuronCores.

Sharding: data + head parallel. 16 (batch, head) pairs -> 2 heads per core
(same batch). Each core computes q/k/v projections for its heads, the
[L, L] logits^T (k on partitions, q on free) via 4-way PE row-tiling with
zero-padded d=8 contractions, exp + per-k sums on the scalar engine
(softmax over q == free axis), folds the softmax normalizer into v, and
accumulates o = v' @ exp in PSUM over key chunks. The final 1x1 output
projection produces a per-core partial which is AllReduce'd over the 4
cores sharing a batch.

Self-contained: shapes/sharding hardcoded for x[2,64,48,48], 8 heads.
"""

import numpy as np

import concourse.bacc as bacc
import concourse.bass as bass
import concourse.mybir as mybir
import concourse.tile as tile
from concourse.bass_utils import run_bass_kernel_spmd

F32 = mybir.dt.float32
BF16 = mybir.dt.bfloat16

B, C, HH, WW = 2, 64, 48, 48
L = HH * WW                   # 2304
NH, DH = 8, 8
NCORES = 8
HPC = 2                       # heads per core
KC = 18                       # key chunks of 128
QT = [(0, 512), (512, 512), (1024, 512), (1536, 512), (2048, 256)]

# Precision plan: projections (q/k/v, final out) run fp32 matmuls (exact,
# keeps the graded q output at fp32 fidelity); the two large matmuls
# (logits, o) run bf16 for PE speed.


def _mm(ap):
    return ap


def build_nc():
    nc = bacc.Bacc(num_devices=NCORES)

    x_d = nc.declare_dram_parameter("x", [C, L], F32, isOutput=False)
    wq_d = nc.declare_dram_parameter("wq_rep", [C, 256], F32, isOutput=False)
    wk_d = nc.declare_dram_parameter("wk_rep", [C, 256], F32, isOutput=False)
    wv_d = nc.declare_dram_parameter("wv_pad", [C, 64], F32, isOutput=False)
    wo_d = nc.declare_dram_parameter("wo_rep", [64, 64], F32, isOutput=False)
    bqk_d = nc.declare_dram_parameter("bias_qk", [128, 4], F32, isOutput=False)
    bv_d = nc.declare_dram_parameter("bias_v", [128, 64], F32, isOutput=False)
    bo_d = nc.declare_dram_parameter("bias_o", [64, 1], F32, isOutput=False)

    q_out_d = nc.declare_dram_parameter("q_part", [16, L], F32, isOutput=True)
    out_d = nc.declare_dram_parameter("out_part", [64, L], F32, isOutput=True)

    out_partial_d = nc.dram_tensor("out_partial", [64, L], F32)
    out_shared_d = nc.dram_tensor("out_shared", [64, L], F32)

    with tile.TileContext(nc, num_cores=NCORES) as tc:
        with tc.tile_pool(name="const", bufs=1) as const_pool:
            x_sb = const_pool.tile([C, L], F32)
            nc.sync.dma_start(out=x_sb, in_=x_d[:, :])
            wq_sb = const_pool.tile([C, 256], F32)
            nc.sync.dma_start(out=wq_sb, in_=wq_d[:, :])
            wk_sb = const_pool.tile([C, 256], F32)
            nc.sync.dma_start(out=wk_sb, in_=wk_d[:, :])
            wv_sb = const_pool.tile([C, 64], F32)
            nc.sync.dma_start(out=wv_sb, in_=wv_d[:, :])
            wo_sb = const_pool.tile([64, 64], F32)
            nc.sync.dma_start(out=wo_sb, in_=wo_d[:, :])
            bqk_sb = const_pool.tile([128, 4], F32)
            nc.sync.dma_start(out=bqk_sb, in_=bqk_d[:, :])
            bv_sb = const_pool.tile([128, 64], F32)
            nc.sync.dma_start(out=bv_sb, in_=bv_d[:, :])
            bo_sb = const_pool.tile([64, 1], F32)
            nc.sync.dma_start(out=bo_sb, in_=bo_d[:, :])

            # Persistent SBUF planes
            qrep_sb = []   # per head: [128, L] replicated q (rows 32g+j), bf16
            kw_sb = []     # per head: [128, L] replicated k, bf16
            qout_sb = []   # per head: [8, L] fp32 exact q rows for the q output
            with tc.tile_pool(name="planes", bufs=1) as planes:
                for hh in range(HPC):
                    t = planes.tile([128, L], BF16, tag=f"qrep{hh}", name=f"qrep{hh}")
                    qrep_sb.append(t)
                    t = planes.tile([128, L], BF16, tag=f"kw{hh}", name=f"kw{hh}")
                    kw_sb.append(t)
                    t = planes.tile([8, L], F32, tag=f"qout{hh}", name=f"qout{hh}")
                    qout_sb.append(t)
                vt_sb = planes.tile([128, KC * 64], F32, tag="vt", name="vt")
                o_sb = planes.tile([64, L], F32, tag="osb", name="osb")
                out_sb = planes.tile([64, L], F32, tag="outsb", name="outsb")

                # ---- Phase 0: projections ----
                with tc.tile_pool(name="p0psum", bufs=2, space="PSUM") as p0:
                    # q and k replicated planes
                    for hh in range(HPC):
                        for (q0, qw) in QT:
                            qp = p0.tile([128, 512], F32, tag="proj", name="qp")
                            nc.tensor.matmul(
                                qp[:, :qw],
                                lhsT=_mm(wq_sb[:, 128 * hh:128 * hh + 128]),
                                rhs=_mm(x_sb[:, q0:q0 + qw]),
                                start=True, stop=True)
                            nc.vector.tensor_scalar_add(
                                out=qrep_sb[hh][:, q0:q0 + qw],
                                in0=qp[:, :qw],
                                scalar1=bqk_sb[:, hh:hh + 1])
                            nc.vector.tensor_scalar_add(
                                out=qout_sb[hh][:, q0:q0 + qw],
                                in0=qp[0:8, :qw],
                                scalar1=bqk_sb[0:8, hh:hh + 1])
                            kp = p0.tile([128, 512], F32, tag="proj", name="kp")
                            nc.tensor.matmul(
                                kp[:, :qw],
                                lhsT=_mm(wk_sb[:, 128 * hh:128 * hh + 128]),
                                rhs=_mm(x_sb[:, q0:q0 + qw]),
                                start=True, stop=True)
                            nc.vector.tensor_scalar_add(
                                out=kw_sb[hh][:, q0:q0 + qw],
                                in0=kp[:, :qw],
                                scalar1=bqk_sb[:, 2 + hh:3 + hh])
                    # vT planes: [128(k), 64] per key chunk
                    for kc in range(KC):
                        vp = p0.tile([128, 64], F32, tag="vtp", name="vp")
                        nc.tensor.matmul(
                            vp,
                            lhsT=_mm(x_sb[:, 128 * kc:128 * kc + 128]),
                            rhs=_mm(wv_sb),
                            start=True, stop=True)
                        nc.vector.tensor_tensor(
                            out=vt_sb[:, 64 * kc:64 * kc + 64],
                            in0=vp, in1=bv_sb,
                            op=mybir.AluOpType.add)

                # q output DMA (exact fp32 rows)
                for hh in range(HPC):
                    nc.sync.dma_start(out=q_out_d[8 * hh:8 * hh + 8, :],
                                      in_=qout_sb[hh][:, :])

                # ---- Main loop: logits -> exp -> o accumulation ----
                with tc.tile_pool(name="mainpsum", bufs=1, space="PSUM") as mp, \
                     tc.tile_pool(name="expp", bufs=3) as expp, \
                     tc.tile_pool(name="small", bufs=4) as small:
                    for hh in range(HPC):
                        o_ps = [mp.tile([64, qw], F32, tag=f"o{t}", name=f"o{t}")
                                for t, (q0, qw) in enumerate(QT)]
                        for kc in range(KC):
                            g = kc % 4
                            expst = expp.tile([128, L], BF16, tag="expst",
                                              name="expst", bufs=3)
                            sums = small.tile([128, 8], F32, tag="sums", name="sums")
                            for t, (q0, qw) in enumerate(QT):
                                lg = mp.tile([128, 512], F32, tag="lg", name="lg",
                                             bufs=3)
                                nc.tensor.matmul(
                                    lg[:, :qw],
                                    lhsT=_mm(kw_sb[hh][32 * g:32 * g + 32,
                                                       128 * kc:128 * kc + 128]),
                                    rhs=_mm(qrep_sb[hh][32 * g:32 * g + 32,
                                                        q0:q0 + qw]),
                                    start=True, stop=True,
                                    tile_position=(32 * g, 0))
                                nc.scalar.activation(
                                    out=expst[:, q0:q0 + qw],
                                    in_=lg[:, :qw],
                                    func=mybir.ActivationFunctionType.Exp,
                                    accum_out=sums[:, t:t + 1])
                            ssum = small.tile([128, 1], F32, tag="ssum", name="ssum")
                            nc.vector.reduce_sum(ssum, sums[:, 0:len(QT)],
                                                 axis=mybir.AxisListType.X)
                            recip = small.tile([128, 1], F32, tag="recip",
                                               name="recip")
                            nc.vector.reciprocal(recip, ssum)
                            vts = small.tile([128, 32], BF16, tag="vts", name="vts")
                            nc.vector.tensor_scalar_mul(
                                out=vts,
                                in0=vt_sb[:, 64 * kc + 32 * hh:64 * kc + 32 * hh + 32],
                                scalar1=recip)
                            for t, (q0, qw) in enumerate(QT):
                                nc.tensor.matmul(
                                    o_ps[t][32 * hh:32 * hh + 32, :],
                                    lhsT=_mm(vts),
                                    rhs=_mm(expst[:, q0:q0 + qw]),
                                    start=(kc == 0), stop=(kc == KC - 1),
                                    tile_position=(0, 32 * hh))
                        # evacuate this head's o rows to SBUF
                        for t, (q0, qw) in enumerate(QT):
                            nc.vector.tensor_copy(
                                out=o_sb[32 * hh:32 * hh + 32, q0:q0 + qw],
                                in_=o_ps[t][32 * hh:32 * hh + 32, :])

                # ---- Final projection ----
                with tc.tile_pool(name="fpsum", bufs=2, space="PSUM") as fp:
                    for (q0, qw) in QT:
                        op = fp.tile([64, 512], F32, tag="fo", name="op")
                        nc.tensor.matmul(
                            op[:, :qw],
                            lhsT=_mm(wo_sb),
                            rhs=_mm(o_sb[:, q0:q0 + qw]),
                            start=True, stop=True)
                        nc.vector.tensor_scalar_add(
                            out=out_sb[:, q0:q0 + qw],
                            in0=op[:, :qw],
                            scalar1=bo_sb)

                nc.sync.dma_start(out=out_partial_d[:, :], in_=out_sb)
                nc.gpsimd.collective_compute(
                    "AllReduce",
                    mybir.AluOpType.add,
                    replica_groups=[[0, 1, 2, 3], [4, 5, 6, 7]],
                    ins=[out_partial_d[:, :]],
                    outs=[out_shared_d[:, :]],
                )
                nc.sync.dma_start(out=out_d[:, :], in_=out_shared_d[:, :])

    nc.compile()
    return nc


def make_core_inputs(core, x, Wq, bq, Wk, bk, Wv, bv, Wo, bo):
    b = core // 4
    base = 16 * (core % 4)
    scale = np.float32(DH ** -0.5)

    x_flat = np.ascontiguousarray(x[b].reshape(C, L)).astype(np.float32)

    wq_rep = np.zeros((C, 256), np.float32)
    wk_rep = np.zeros((C, 256), np.float32)
    bias_qk = np.zeros((128, 4), np.float32)
    for hh in range(HPC):
        for g in range(4):
            cols = 128 * hh + 32 * g
            wq_rep[:, cols:cols + 8] = (Wq[base + 8 * hh:base + 8 * hh + 8] * scale).T
            wk_rep[:, cols:cols + 8] = Wk[base + 8 * hh:base + 8 * hh + 8].T
            bias_qk[32 * g:32 * g + 8, hh] = bq[base + 8 * hh:base + 8 * hh + 8] * scale
            bias_qk[32 * g:32 * g + 8, 2 + hh] = bk[base + 8 * hh:base + 8 * hh + 8]

    wv_pad = np.zeros((C, 64), np.float32)
    bias_v = np.zeros((128, 64), np.float32)
    wo_rep = np.zeros((64, 64), np.float32)
    for hh in range(HPC):
        wv_pad[:, 32 * hh:32 * hh + 8] = Wv[base + 8 * hh:base + 8 * hh + 8].T
        bias_v[:, 32 * hh:32 * hh + 8] = bv[base + 8 * hh:base + 8 * hh + 8][None, :]
        wo_rep[32 * hh:32 * hh + 8, :] = Wo[:, base + 8 * hh:base + 8 * hh + 8].T

    bias_o = (bo / 4.0).astype(np.float32).reshape(64, 1)

    return dict(x=x_flat, wq_rep=wq_rep, wk_rep=wk_rep, wv_pad=wv_pad,
                wo_rep=wo_rep, bias_qk=bias_qk, bias_v=bias_v, bias_o=bias_o)


def assemble_outputs(results):
    out_full = np.zeros((B, 64, L), np.float32)
    q_full = np.zeros((B, 64, L), np.float32)
    for core in range(NCORES):
        b = core // 4
        base = 16 * (core % 4)
        q_full[b, base:base + 16] = results[core]["q_part"]
    out_full[0] = results[0]["out_part"]
    out_full[1] = results[4]["out_part"]
    return (out_full.reshape(B, 64, HH, WW), q_full.reshape(B, 64, HH, WW))


_NC_CACHE = {}


def get_nc():
    if "nc" not in _NC_CACHE:
        _NC_CACHE["nc"] = build_nc()
    return _NC_CACHE["nc"]


def kernel(**inputs):
    inputs = {k: np.asarray(v) for k, v in inputs.items()}
    nc = get_nc()
    in_maps = [make_core_inputs(c, **inputs) for c in range(NCORES)]
    res = run_bass_kernel_spmd(nc, in_maps, core_ids=list(range(NCORES)))
    return assemble_outputs(res.results)


if __name__ == "__main__":
    import reference
    inputs = {k: np.asarray(v) for k, v in reference.setup_inputs().items()}
    out, q = kernel(**inputs)
    ref_out, ref_q = [np.asarray(v) for v in reference.reference(**inputs)]
    for name, got, want in [("out", out, ref_out), ("q", q, ref_q)]:
        err = np.abs(got - want).max() / np.abs(want).max()
        print(f"{name}: absmax-rel err = {err:.3e}")
